# revision 37
# baseline (speedup 1.0000x reference)
"""Trainium2 Bass kernel for a 6-level db4 DWT (zero-padding mode).

Input x: [64, 262144] f32. Output (cA6, cD6, cD5, cD4, cD3, cD2, cD1).

Strategy (per NeuronCore, batch sharded 8 rows/core):
  - Signal stored partition-major in SBUF: col j holds a[128j .. 128j+128) down
    the 128 partitions.  Built from a free-major DMA load via PE transposes.
  - One DWT level = banded matmuls: each 128-coeff output segment s' reads
    input cols 2s'-1 (6-row halo), 2s', 2s'+1; three accumulating fp32r
    matmuls per filter into PSUM.  cA lands partition-major (next level's
    input layout), cD is PE-transposed back to [segment, coeff] layout so the
    DMA to DRAM moves 512B-contiguous chunks per partition.
  - Levels 1-3 are processed per batch row; levels 4-6 pack all rows into one
    tile so matmul free dims stay >= 256 (fp32r fast path).
"""

import sys

sys.path.insert(0, "/opt/trn_rl_repo")

import numpy as np

import concourse.bacc as bacc
import concourse.mybir as mybir
import concourse.tile as tile
from concourse.masks import make_identity

F32 = mybir.dt.float32
F32R = mybir.dt.float32r

N0 = 262144
B_FULL = 64
N_CORES = 8
ROWS = B_FULL // N_CORES  # 8 rows per core
LEVEL = 6
F = 8

DEC_LO = np.array([-0.010597401784997278, 0.032883011666982945, 0.030841381835986965,
                   -0.18703481171888114, -0.02798376941698385, 0.6308807679295904,
                   0.7148465705525415, 0.23037781330885523], dtype=np.float64)
DEC_HI = np.array([-0.23037781330885523, 0.7148465705525415, -0.6308807679295904,
                   -0.02798376941698385, 0.18703481171888114, 0.030841381835986965,
                   -0.032883011666982945, -0.010597401784997278], dtype=np.float64)
W_LO = DEC_LO[::-1]
W_HI = DEC_HI[::-1]


def _ceil(a, b):
    return -(-a // b)


def make_band_arrays():
    """Host-side constant arrays: wmain [128, 512] = [A2|A3|D2|D3],
    whalo [6, 256] = [A1|D1].  out i = 128 s' + c, taps at u = 2c + j - 6
    relative to base 256 s'; u in [-6,0) -> col 2s'-1 rows 122+u+6,
    u in [0,128) -> col 2s' row u, u in [128,256) -> col 2s'+1 row u-128."""
    mats = {}
    for name, w in (("A", W_LO), ("D", W_HI)):
        m1 = np.zeros((6, 128), np.float32)
        m2 = np.zeros((128, 128), np.float32)
        m3 = np.zeros((128, 128), np.float32)
        for c in range(128):
            for j in range(F):
                u = 2 * c + j - 6
                if u < 0:
                    m1[u + 6, c] = w[j]
                elif u < 128:
                    m2[u, c] = w[j]
                else:
                    m3[u - 128, c] = w[j]
        mats[name] = (m1, m2, m3)
    wmain = np.concatenate(
        [mats["A"][1], mats["A"][2], mats["D"][1], mats["D"][2]], axis=1)
    # halo matrices as full K=128 contraction: only rows 122..127 carry data
    # (zeros elsewhere), so the halo matmul uses the same base partition 0 and
    # K=128 as the main matmuls.
    whalo = np.zeros((128, 256), np.float32)
    whalo[122:128, 0:128] = mats["A"][0]
    whalo[122:128, 128:256] = mats["D"][0]
    return wmain, whalo


def plan_levels(n0=N0):
    """Per-level geometry."""
    levels = []
    n = n0
    for _ in range(LEVEL):
        c = _ceil(n, 128)          # data cols of the input signal
        m = (n + F - 1) // 2       # output length
        s = _ceil(m, 128)          # output segments (=input data cols of next)
        sp = s + (s % 2)           # padded to even (fp32r needs even mm counts)
        w = 2 * sp + 2             # tile width per row: lead zero + 2sp cols + pad
        nb = _ceil(s, 128)         # 128-seg transpose blocks for the cD output
        levels.append(dict(n=n, c=c, m=m, s=s, sp=sp, w=w, nb=nb))
        n = m
    return levels


def chunk_sizes(sp, cap=512):
    """Split even `sp` into even chunks <= cap."""
    assert sp % 2 == 0
    half = sp // 2
    nch = max(1, _ceil(half, cap // 2))
    base, rem = divmod(half, nch)
    return [2 * (base + (1 if i < rem else 0)) for i in range(nch)]


def row_chunks(rows, sp, cap=512):
    """Split `rows` into groups so group*sp <= cap."""
    per = max(1, cap // sp)
    nch = _ceil(rows, per)
    base, rem = divmod(rows, nch)
    return [base + (1 if i < rem else 0) for i in range(nch)]


OUT_NAMES = ["a6", "d6", "d5", "d4", "d3", "d2", "d1"]


def build_nc(rows=ROWS, n0=N0):
    levels = plan_levels(n0)
    nc = bacc.Bacc(None, target_bir_lowering=False)

    x_in = nc.declare_dram_parameter("x", [rows, n0], F32, isOutput=False)
    wm_in = nc.declare_dram_parameter("wmain", [128, 512], F32, isOutput=False)
    wh_in = nc.declare_dram_parameter("whalo", [128, 256], F32, isOutput=False)

    # outputs: cD per level + cA6
    d_out = [nc.declare_dram_parameter(f"d{l + 1}", [rows, levels[l]["m"]], F32,
                                       isOutput=True) for l in range(LEVEL)]
    a6_out = nc.declare_dram_parameter("a6", [rows, levels[-1]["m"]], F32,
                                       isOutput=True)

    with tile.TileContext(nc) as tc:
        with (
            tc.tile_pool(name="consts", bufs=1) as consts,
            tc.tile_pool(name="fpool", bufs=2) as fpool,
            tc.tile_pool(name="inp", bufs=2) as inp,          # per-row IN tiles L1-3
            tc.tile_pool(name="packed", bufs=1) as packed,    # IN/SD/OUT for L4-6
            tc.tile_pool(name="sd", bufs=2) as sdp,           # per-row SD staging
            tc.tile_pool(name="outp", bufs=2) as outp,        # per-row OUT staging
            tc.tile_pool(name="pa", bufs=2, space="PSUM") as pa,
            tc.tile_pool(name="pd", bufs=2, space="PSUM") as pd,
            tc.tile_pool(name="pt", bufs=2, space="PSUM") as pt,
            tc.tile_pool(name="px", bufs=2, space="PSUM") as px,
        ):
            ident = consts.tile([128, 128], F32, tag="ident")
            make_identity(nc, ident)
            wm = consts.tile([128, 512], F32, tag="wm")
            wh = consts.tile([128, 256], F32, tag="wh")
            wm_stage = consts.tile([128, 512], F32, tag="wms")
            wh_stage = consts.tile([128, 256], F32, tag="whs")
            nc.sync.dma_start(out=wm_stage[:], in_=wm_in[:])
            nc.sync.dma_start(out=wh_stage[:], in_=wh_in[:])
            # fp32r-rounded zero source (memset can't emit fp32r directly)
            zsrc = consts.tile([128, 256], F32, tag="zsrc")
            nc.gpsimd.memset(zsrc[:], 0.0)
            # round-copy so the fp32r matmul consumers see fp32r-rounded data
            nc.scalar.copy(wm[:].bitcast(F32R), wm_stage[:])
            nc.scalar.copy(wh[:].bitcast(F32R), wh_stage[:])

            def zero_cols(dst_ap):
                """Write fp32r-rounded zeros into a [128, n<=256] region."""
                n = dst_ap.shape[-1]
                nc.vector.tensor_copy(dst_ap.bitcast(F32R), zsrc[:, 0:n])

            lhs = {  # (halo, main, odd) per filter
                "A": (wh[:, 0:128], wm[:, 0:128], wm[:, 128:256]),
                "D": (wh[:, 128:256], wm[:, 256:384], wm[:, 384:512]),
            }

            def conv_chunk(in_pairs, psum_t, filt, s0, ns, extra=None):
                """3 accumulating fp32r matmuls for out segs [s0, s0+ns).
                in_pairs: AP [..., s, two] pair view (row dim optional via
                `extra` = (r0, nr) selecting rows of a packed pair view)."""
                m1, m2, m3 = lhs[filt]
                if extra is None:
                    ra = in_pairs[:, s0:s0 + ns, 0]
                    rb = in_pairs[:, s0:s0 + ns, 1]
                    rc = in_pairs[:, s0 + 1:s0 + 1 + ns, 0]
                else:
                    r0, nr = extra
                    ra = in_pairs[:, r0:r0 + nr, s0:s0 + ns, 0]
                    rb = in_pairs[:, r0:r0 + nr, s0:s0 + ns, 1]
                    rc = in_pairs[:, r0:r0 + nr, s0 + 1:s0 + 1 + ns, 0]
                nc.tensor.matmul(psum_t, m1.bitcast(F32R), ra.bitcast(F32R),
                                 start=True, stop=False)
                nc.tensor.matmul(psum_t, m2.bitcast(F32R), rb.bitcast(F32R),
                                 start=False, stop=False)
                nc.tensor.matmul(psum_t, m3.bitcast(F32R), rc.bitcast(F32R),
                                 start=False, stop=True)

            def emit_out(sd_t, out_t, lv, base=0):
                """PE-transpose SD [128, 128*nb] -> OUT (seg-major)."""
                for b in range(lv["nb"]):
                    ptile = pt.tile([128, 128], F32, tag="pt")
                    nc.tensor.transpose(
                        ptile[:], sd_t[:, base + 128 * b:base + 128 * (b + 1)],
                        ident[:])
                    nc.vector.tensor_copy(
                        out_t[:, base + 128 * b:base + 128 * (b + 1)], ptile[:])

            def dma_out(out_t, dram_t, r, lv, base_col=0):
                """DMA one row's output coeffs from seg-major OUT tile."""
                m = lv["m"]
                full_segs = m // 128
                nbf = full_segs // 128
                tail = m - 128 * full_segs
                if nbf:
                    dst = dram_t[r, 0:16384 * nbf].rearrange(
                        "(b j c) -> j b c", j=128, c=128)
                    src = out_t[:, base_col:base_col + 128 * nbf].rearrange(
                        "p (b c) -> p b c", c=128)
                    nc.sync.dma_start(out=dst, in_=src)
                rem = full_segs - 128 * nbf
                if rem:
                    dst = dram_t[r, 16384 * nbf:16384 * nbf + 128 * rem].rearrange(
                        "(j c) -> j c", c=128)
                    src = out_t[0:rem, base_col + 128 * nbf:base_col + 128 * (nbf + 1)]
                    nc.sync.dma_start(out=dst, in_=src)
                if tail:
                    p_t = full_segs % 128
                    b_t = full_segs // 128
                    dst = dram_t[r:r + 1, 128 * full_segs:m]
                    src = out_t[p_t:p_t + 1,
                                base_col + 128 * b_t:base_col + 128 * b_t + tail]
                    nc.sync.dma_start(out=dst, in_=src)

            # ---------- per-row levels 1..3 ----------
            lv4 = levels[3]
            in4 = packed.tile([128, rows * lv4["w"]], F32, tag="in4")
            in4_rows = in4.rearrange("p (r w) -> p r w", r=rows)

            for r in range(rows):
                # load x row free-major: F[p, 128*tau + f] = x[r, 16384*tau + 128*p + f]
                ftile = fpool.tile([128, n0 // 128], F32, tag="f")
                ntau = n0 // 16384
                src = x_in[r].rearrange("(t p f) -> p t f", p=128, f=128)
                nc.sync.dma_start(
                    out=ftile.rearrange("p (t f) -> p t f", f=128), in_=src)

                lv1 = levels[0]
                in1 = inp.tile([128, lv1["w"]], F32, tag="in1")
                zero_cols(in1[:, 0:1])
                zero_cols(in1[:, 1 + lv1["c"]:lv1["w"]])
                for t in range(ntau):
                    ptile = px.tile([128, 128], F32, tag="px")
                    nc.tensor.transpose(ptile[:], ftile[:, 128 * t:128 * (t + 1)],
                                        ident[:])
                    nc.vector.tensor_copy(
                        in1[:, 1 + 128 * t:1 + 128 * (t + 1)].bitcast(F32R),
                        ptile[:])

                cur = in1
                for li in range(3):
                    lv = levels[li]
                    nxt_lv = levels[li + 1]
                    if li < 2:
                        nxt = inp.tile([128, nxt_lv["w"]], F32,
                                       tag=f"in{li + 2}")
                        zero_cols(nxt[:, 0:1])
                        zero_cols(nxt[:, 1 + nxt_lv["c"]:nxt_lv["w"]])
                        nxt_dst = nxt
                        dst_off = 1
                    else:
                        nxt = in4
                        nxt_dst = None  # write via in4_rows
                    sd_t = sdp.tile([128, 128 * lv["nb"]], F32, tag=f"sd{li + 1}")
                    if 128 * lv["nb"] > lv["s"]:
                        nc.gpsimd.memset(sd_t[:, lv["s"]:128 * lv["nb"]], 0.0)
                    pairs = cur.rearrange("p (s two) -> p s two", two=2)
                    s0 = 0
                    for ns in chunk_sizes(lv["sp"]):
                        pa_t = pa.tile([128, 512], F32, tag="pa")
                        pd_t = pd.tile([128, 512], F32, tag="pd")
                        conv_chunk(pairs, pa_t[:, 0:ns], "A", s0, ns)
                        conv_chunk(pairs, pd_t[:, 0:ns], "D", s0, ns)
                        nsv = min(ns, lv["s"] - s0)  # skip the even-pad segment
                        if li < 2:
                            nc.scalar.copy(
                                nxt_dst[:, dst_off + s0:dst_off + s0 + nsv]
                                .bitcast(F32R), pa_t[:, 0:nsv])
                        else:
                            nc.scalar.copy(
                                in4_rows[:, r, 1 + s0:1 + s0 + nsv].bitcast(F32R),
                                pa_t[:, 0:nsv])
                        nc.scalar.copy(sd_t[:, s0:s0 + nsv], pd_t[:, 0:nsv])
                        s0 += ns
                    out_t = outp.tile([128, 128 * lv["nb"]], F32, tag=f"out{li + 1}")
                    emit_out(sd_t, out_t, lv)
                    dma_out(out_t, d_out[li], r, lv)
                    cur = nxt
                # zero-cols for this row's region of in4
                zero_cols(in4[:, r * lv4["w"]:r * lv4["w"] + 1])
                zero_cols(in4[:, r * lv4["w"] + 1 + lv4["c"]:(r + 1) * lv4["w"]])

            # ---------- packed levels 4..6 ----------
            cur = in4
            for li in range(3, LEVEL):
                lv = levels[li]
                last = li == LEVEL - 1
                if not last:
                    nxt_lv = levels[li + 1]
                    nxt = packed.tile([128, rows * nxt_lv["w"]], F32,
                                      tag=f"in{li + 2}")
                    nxt_rows = nxt.rearrange("p (r w) -> p r w", r=rows)
                    for r in range(rows):
                        w_ = nxt_lv["w"]
                        zero_cols(nxt[:, r * w_:r * w_ + 1])
                        zero_cols(nxt[:, r * w_ + 1 + nxt_lv["c"]:(r + 1) * w_])
                sd_t = packed.tile([128, rows * 128 * lv["nb"]], F32,
                                   tag=f"sdp{li}")
                sda_t = None
                if last:
                    sda_t = packed.tile([128, rows * 128 * lv["nb"]], F32,
                                        tag="sdpa")
                if 128 * lv["nb"] > lv["s"]:
                    for r in range(rows):
                        gap = slice(r * 128 * lv["nb"] + lv["s"],
                                    (r + 1) * 128 * lv["nb"])
                        nc.gpsimd.memset(sd_t[:, gap], 0.0)
                        if last:
                            nc.gpsimd.memset(sda_t[:, gap], 0.0)
                pairs = cur.rearrange("p (r s two) -> p r s two", r=rows, two=2)
                r0 = 0
                for nr in row_chunks(rows, lv["sp"]):
                    ns_tot = nr * lv["sp"]
                    pa_t = pa.tile([128, 512], F32, tag="pa")
                    pd_t = pd.tile([128, 512], F32, tag="pd")
                    conv_chunk(pairs, pa_t[:, 0:ns_tot], "A", 0, lv["sp"],
                               extra=(r0, nr))
                    conv_chunk(pairs, pd_t[:, 0:ns_tot], "D", 0, lv["sp"],
                               extra=(r0, nr))
                    for i in range(nr):
                        r = r0 + i
                        sl = pd_t[:, i * lv["sp"]:i * lv["sp"] + lv["s"]]
                        nc.scalar.copy(
                            sd_t[:, r * 128 * lv["nb"]:r * 128 * lv["nb"] + lv["s"]],
                            sl)
                        sla = pa_t[:, i * lv["sp"]:i * lv["sp"] + lv["s"]]
                        if last:
                            nc.scalar.copy(
                                sda_t[:, r * 128 * lv["nb"]:
                                      r * 128 * lv["nb"] + lv["s"]], sla)
                        else:
                            nc.scalar.copy(
                                nxt_rows[:, r, 1:1 + lv["s"]].bitcast(F32R), sla)
                    r0 += nr
                out_t = packed.tile([128, rows * 128 * lv["nb"]], F32,
                                    tag=f"outp{li}")
                outa_t = None
                if last:
                    outa_t = packed.tile([128, rows * 128 * lv["nb"]], F32,
                                         tag="outpa")
                for r in range(rows):
                    base = r * 128 * lv["nb"]
                    emit_out(sd_t, out_t, lv, base=base)
                    if last:
                        emit_out(sda_t, outa_t, lv, base=base)
                    dma_out(out_t, d_out[li], r, lv, base_col=base)
                    if last:
                        dma_out(outa_t, a6_out, r, lv, base_col=base)
                if not last:
                    cur = nxt
    nc.compile()
    return nc


_CACHE = {}


def _get_nc():
    if "nc" not in _CACHE:
        _CACHE["nc"] = build_nc()
        _CACHE["w"] = make_band_arrays()
    return _CACHE["nc"], _CACHE["w"]


LAST_RESULT = None


def kernel(x):
    global LAST_RESULT
    x = np.ascontiguousarray(np.asarray(x), dtype=np.float32)
    assert x.shape == (B_FULL, N0)
    from concourse.bass_utils import run_bass_kernel_spmd

    nc, (wmain, whalo) = _get_nc()
    in_maps = [
        {"x": x[c * ROWS:(c + 1) * ROWS], "wmain": wmain, "whalo": whalo}
        for c in range(N_CORES)
    ]
    res = run_bass_kernel_spmd(nc, in_maps, core_ids=list(range(N_CORES)))
    LAST_RESULT = res
    outs = []
    for name in OUT_NAMES:
        outs.append(np.concatenate([res.results[c][name]
                                    for c in range(N_CORES)], axis=0))
    return tuple(outs)


# revision 38
# speedup vs baseline: 1.2773x; 1.2773x over previous
"""Trainium2 Bass kernel for a 6-level db4 DWT (zero-padding mode).

Input x: [64, 262144] f32. Output (cA6, cD6, cD5, cD4, cD3, cD2, cD1).

Strategy (per NeuronCore, batch sharded 8 rows/core):
  - Signal stored partition-major in SBUF: col j holds a[128j .. 128j+128) down
    the 128 partitions.  Built from a free-major cast-to-fp16 DMA load via PE
    transposes (fp16 transposes run at 1 cycle/row).
  - One DWT level = banded matmuls in fp16 (fp32 PSUM accumulation): each
    128-coeff output segment s' reads input cols 2s'-1 (6-row halo), 2s',
    2s'+1; three accumulating matmuls per filter into PSUM.  cA lands
    partition-major (next level's input layout, cast back to fp16), cD is
    PE-transposed to [segment, coeff] layout so the fp32 DMA to DRAM moves
    512B-contiguous chunks per partition.
  - Levels 1-3 are processed per batch row; levels 4-6 pack all rows into one
    tile so matmul free dims stay large.
"""

import sys

sys.path.insert(0, "/opt/trn_rl_repo")

import numpy as np

import concourse.bacc as bacc
import concourse.mybir as mybir
import concourse.tile as tile
from concourse.masks import make_identity

F32 = mybir.dt.float32
F16 = mybir.dt.float16

N0 = 262144
B_FULL = 64
N_CORES = 8
ROWS = B_FULL // N_CORES  # 8 rows per core
LEVEL = 6
F = 8

DEC_LO = np.array([-0.010597401784997278, 0.032883011666982945, 0.030841381835986965,
                   -0.18703481171888114, -0.02798376941698385, 0.6308807679295904,
                   0.7148465705525415, 0.23037781330885523], dtype=np.float64)
DEC_HI = np.array([-0.23037781330885523, 0.7148465705525415, -0.6308807679295904,
                   -0.02798376941698385, 0.18703481171888114, 0.030841381835986965,
                   -0.032883011666982945, -0.010597401784997278], dtype=np.float64)
W_LO = DEC_LO[::-1]
W_HI = DEC_HI[::-1]


def _ceil(a, b):
    return -(-a // b)


def make_band_arrays():
    """Host-side constant arrays (fp16): wmain [128, 512] = [A2|A3|D2|D3],
    whalo [128, 256] = [A1|D1] with data only in rows 122..127.
    out i = 128 s' + c, taps at u = 2c + j - 6 relative to base 256 s';
    u in [-6,0) -> col 2s'-1 row 122+u+6, u in [0,128) -> col 2s' row u,
    u in [128,256) -> col 2s'+1 row u-128."""
    mats = {}
    for name, w in (("A", W_LO), ("D", W_HI)):
        m1 = np.zeros((128, 128), np.float32)
        m2 = np.zeros((128, 128), np.float32)
        m3 = np.zeros((128, 128), np.float32)
        for c in range(128):
            for j in range(F):
                u = 2 * c + j - 6
                if u < 0:
                    m1[122 + u + 6, c] = w[j]
                elif u < 128:
                    m2[u, c] = w[j]
                else:
                    m3[u - 128, c] = w[j]
        mats[name] = (m1, m2, m3)
    wmain = np.concatenate(
        [mats["A"][1], mats["A"][2], mats["D"][1], mats["D"][2]], axis=1)
    whalo = np.concatenate([mats["A"][0], mats["D"][0]], axis=1)
    return wmain.astype(np.float16), whalo.astype(np.float16)


def plan_levels(n0=N0):
    levels = []
    n = n0
    for _ in range(LEVEL):
        c = _ceil(n, 128)          # data cols of the input signal
        m = (n + F - 1) // 2       # output length
        s = _ceil(m, 128)          # output segments (=input data cols of next)
        sp = s + (s % 2)           # padded to even
        w = 2 * sp + 2             # tile width per row
        nb = _ceil(s, 128)         # 128-seg transpose blocks for outputs
        levels.append(dict(n=n, c=c, m=m, s=s, sp=sp, w=w, nb=nb))
        n = m
    return levels


def chunk_sizes(sp, cap=512):
    """Split even `sp` into even chunks <= cap."""
    assert sp % 2 == 0
    half = sp // 2
    nch = max(1, _ceil(half, cap // 2))
    base, rem = divmod(half, nch)
    return [2 * (base + (1 if i < rem else 0)) for i in range(nch)]


def row_chunks(rows, sp, cap=512):
    """Split `rows` into groups so group*sp <= cap."""
    per = max(1, cap // sp)
    nch = _ceil(rows, per)
    base, rem = divmod(rows, nch)
    return [base + (1 if i < rem else 0) for i in range(nch)]


OUT_NAMES = ["a6", "d6", "d5", "d4", "d3", "d2", "d1"]


def build_nc(rows=ROWS, n0=N0):
    levels = plan_levels(n0)
    nc = bacc.Bacc(None, target_bir_lowering=False)

    x_in = nc.declare_dram_parameter("x", [rows, n0], F32, isOutput=False)
    wm_in = nc.declare_dram_parameter("wmain", [128, 512], F16, isOutput=False)
    wh_in = nc.declare_dram_parameter("whalo", [128, 256], F16, isOutput=False)

    d_out = [nc.declare_dram_parameter(f"d{l + 1}", [rows, levels[l]["m"]], F32,
                                       isOutput=True) for l in range(LEVEL)]
    a6_out = nc.declare_dram_parameter("a6", [rows, levels[-1]["m"]], F32,
                                       isOutput=True)

    with tile.TileContext(nc) as tc:
        with (
            tc.tile_pool(name="consts", bufs=1) as consts,
            tc.tile_pool(name="fpool", bufs=2) as fpool,
            tc.tile_pool(name="inp", bufs=2) as inp,          # per-row IN tiles L1-3
            tc.tile_pool(name="packed", bufs=1) as packed,    # IN/SD/OUT for L4-6
            tc.tile_pool(name="sd", bufs=2) as sdp,           # per-row SD staging
            tc.tile_pool(name="outp", bufs=2) as outp,        # per-row OUT staging
            tc.tile_pool(name="pa", bufs=2, space="PSUM") as pa,
            tc.tile_pool(name="pd", bufs=2, space="PSUM") as pd,
            tc.tile_pool(name="pt", bufs=2, space="PSUM") as pt,
            tc.tile_pool(name="px", bufs=2, space="PSUM") as px,
        ):
            ident = consts.tile([128, 128], F16, tag="ident")
            make_identity(nc, ident)
            wm = consts.tile([128, 512], F16, tag="wm")
            wh = consts.tile([128, 256], F16, tag="wh")
            nc.sync.dma_start(out=wm[:], in_=wm_in[:])
            nc.sync.dma_start(out=wh[:], in_=wh_in[:])
            zsrc = consts.tile([128, 256], F16, tag="zsrc")
            nc.gpsimd.memset(zsrc[:], 0.0)

            def zero_cols(dst_ap):
                n = dst_ap.shape[-1]
                nc.vector.tensor_copy(dst_ap, zsrc[:, 0:n])

            lhs = {  # (halo, main, odd) per filter
                "A": (wh[:, 0:128], wm[:, 0:128], wm[:, 128:256]),
                "D": (wh[:, 128:256], wm[:, 256:384], wm[:, 384:512]),
            }

            def conv_chunk(in_pairs, psum_t, filt, s0, ns, extra=None):
                """3 accumulating fp16 matmuls for out segs [s0, s0+ns)."""
                m1, m2, m3 = lhs[filt]
                if extra is None:
                    ra = in_pairs[:, s0:s0 + ns, 0]
                    rb = in_pairs[:, s0:s0 + ns, 1]
                    rc = in_pairs[:, s0 + 1:s0 + 1 + ns, 0]
                else:
                    r0, nr = extra
                    ra = in_pairs[:, r0:r0 + nr, s0:s0 + ns, 0]
                    rb = in_pairs[:, r0:r0 + nr, s0:s0 + ns, 1]
                    rc = in_pairs[:, r0:r0 + nr, s0 + 1:s0 + 1 + ns, 0]
                nc.tensor.matmul(psum_t, m1, ra, start=True, stop=False)
                nc.tensor.matmul(psum_t, m2, rb, start=False, stop=False)
                nc.tensor.matmul(psum_t, m3, rc, start=False, stop=True)

            def emit_out(sd_t, out_t, lv, base=0):
                """PE-transpose fp16 SD [128, 128*nb] -> fp32 OUT (seg-major)."""
                for b in range(lv["nb"]):
                    ptile = pt.tile([128, 128], F16, tag="pt")
                    nc.tensor.transpose(
                        ptile[:], sd_t[:, base + 128 * b:base + 128 * (b + 1)],
                        ident[:])
                    nc.vector.tensor_copy(
                        out_t[:, base + 128 * b:base + 128 * (b + 1)], ptile[:])

            def dma_out(out_t, dram_t, r, lv, base_col=0):
                """DMA one row's output coeffs from seg-major fp32 OUT tile."""
                m = lv["m"]
                full_segs = m // 128
                nbf = full_segs // 128
                tail = m - 128 * full_segs
                if nbf:
                    dst = dram_t[r, 0:16384 * nbf].rearrange(
                        "(b j c) -> j b c", j=128, c=128)
                    src = out_t[:, base_col:base_col + 128 * nbf].rearrange(
                        "p (b c) -> p b c", c=128)
                    nc.sync.dma_start(out=dst, in_=src)
                rem = full_segs - 128 * nbf
                if rem:
                    dst = dram_t[r, 16384 * nbf:16384 * nbf + 128 * rem].rearrange(
                        "(j c) -> j c", c=128)
                    src = out_t[0:rem, base_col + 128 * nbf:base_col + 128 * (nbf + 1)]
                    nc.sync.dma_start(out=dst, in_=src)
                if tail:
                    p_t = full_segs % 128
                    b_t = full_segs // 128
                    dst = dram_t[r:r + 1, 128 * full_segs:m]
                    src = out_t[p_t:p_t + 1,
                                base_col + 128 * b_t:base_col + 128 * b_t + tail]
                    nc.sync.dma_start(out=dst, in_=src)

            # ---------- per-row levels 1..3 ----------
            lv4 = levels[3]
            in4 = packed.tile([128, rows * lv4["w"]], F16, tag="in4")
            in4_rows = in4.rearrange("p (r w) -> p r w", r=rows)

            for r in range(rows):
                # load x row free-major, casting f32 -> f16 in the DMA (SWDGE)
                ftile = fpool.tile([128, n0 // 128], F16, tag="f")
                ntau = n0 // 16384
                src = x_in[r].rearrange("(t p f) -> p t f", p=128, f=128)
                nc.gpsimd.dma_start(
                    out=ftile.rearrange("p (t f) -> p t f", f=128), in_=src)

                lv1 = levels[0]
                in1 = inp.tile([128, lv1["w"]], F16, tag="in1")
                zero_cols(in1[:, 0:1])
                zero_cols(in1[:, 1 + lv1["c"]:lv1["w"]])
                for t in range(ntau):
                    ptile = px.tile([128, 128], F16, tag="px")
                    nc.tensor.transpose(ptile[:], ftile[:, 128 * t:128 * (t + 1)],
                                        ident[:])
                    nc.vector.tensor_copy(in1[:, 1 + 128 * t:1 + 128 * (t + 1)],
                                          ptile[:])

                cur = in1
                for li in range(3):
                    lv = levels[li]
                    nxt_lv = levels[li + 1]
                    if li < 2:
                        nxt = inp.tile([128, nxt_lv["w"]], F16,
                                       tag=f"in{li + 2}")
                        zero_cols(nxt[:, 0:1])
                        zero_cols(nxt[:, 1 + nxt_lv["c"]:nxt_lv["w"]])
                        nxt_dst = nxt
                    sd_t = sdp.tile([128, 128 * lv["nb"]], F16, tag=f"sd{li + 1}")
                    if 128 * lv["nb"] > lv["s"]:
                        nc.gpsimd.memset(sd_t[:, lv["s"]:128 * lv["nb"]], 0.0)
                    pairs = cur.rearrange("p (s two) -> p s two", two=2)
                    s0 = 0
                    for ns in chunk_sizes(lv["sp"]):
                        pa_t = pa.tile([128, 512], F32, tag="pa")
                        pd_t = pd.tile([128, 512], F32, tag="pd")
                        conv_chunk(pairs, pa_t[:, 0:ns], "A", s0, ns)
                        conv_chunk(pairs, pd_t[:, 0:ns], "D", s0, ns)
                        nsv = min(ns, lv["s"] - s0)  # skip the even-pad segment
                        if li < 2:
                            nc.scalar.copy(nxt_dst[:, 1 + s0:1 + s0 + nsv],
                                           pa_t[:, 0:nsv])
                        else:
                            nc.scalar.copy(in4_rows[:, r, 1 + s0:1 + s0 + nsv],
                                           pa_t[:, 0:nsv])
                        nc.scalar.copy(sd_t[:, s0:s0 + nsv], pd_t[:, 0:nsv])
                        s0 += ns
                    out_t = outp.tile([128, 128 * lv["nb"]], F32, tag=f"out{li + 1}")
                    emit_out(sd_t, out_t, lv)
                    dma_out(out_t, d_out[li], r, lv)
                    cur = nxt
                # zero-cols for this row's region of in4
                zero_cols(in4[:, r * lv4["w"]:r * lv4["w"] + 1])
                zero_cols(in4[:, r * lv4["w"] + 1 + lv4["c"]:(r + 1) * lv4["w"]])

            # ---------- packed levels 4..6 ----------
            cur = in4
            for li in range(3, LEVEL):
                lv = levels[li]
                last = li == LEVEL - 1
                if not last:
                    nxt_lv = levels[li + 1]
                    nxt = packed.tile([128, rows * nxt_lv["w"]], F16,
                                      tag=f"in{li + 2}")
                    nxt_rows = nxt.rearrange("p (r w) -> p r w", r=rows)
                    for r in range(rows):
                        w_ = nxt_lv["w"]
                        zero_cols(nxt[:, r * w_:r * w_ + 1])
                        zero_cols(nxt[:, r * w_ + 1 + nxt_lv["c"]:(r + 1) * w_])
                sd_t = packed.tile([128, rows * 128 * lv["nb"]], F16,
                                   tag=f"sdp{li}")
                sda_t = None
                if last:
                    sda_t = packed.tile([128, rows * 128 * lv["nb"]], F16,
                                        tag="sdpa")
                if 128 * lv["nb"] > lv["s"]:
                    for r in range(rows):
                        gap = slice(r * 128 * lv["nb"] + lv["s"],
                                    (r + 1) * 128 * lv["nb"])
                        nc.gpsimd.memset(sd_t[:, gap], 0.0)
                        if last:
                            nc.gpsimd.memset(sda_t[:, gap], 0.0)
                pairs = cur.rearrange("p (r s two) -> p r s two", r=rows, two=2)
                r0 = 0
                for nr in row_chunks(rows, lv["sp"]):
                    ns_tot = nr * lv["sp"]
                    pa_t = pa.tile([128, 512], F32, tag="pa")
                    pd_t = pd.tile([128, 512], F32, tag="pd")
                    conv_chunk(pairs, pa_t[:, 0:ns_tot], "A", 0, lv["sp"],
                               extra=(r0, nr))
                    conv_chunk(pairs, pd_t[:, 0:ns_tot], "D", 0, lv["sp"],
                               extra=(r0, nr))
                    for i in range(nr):
                        r = r0 + i
                        sl = pd_t[:, i * lv["sp"]:i * lv["sp"] + lv["s"]]
                        nc.scalar.copy(
                            sd_t[:, r * 128 * lv["nb"]:r * 128 * lv["nb"] + lv["s"]],
                            sl)
                        sla = pa_t[:, i * lv["sp"]:i * lv["sp"] + lv["s"]]
                        if last:
                            nc.scalar.copy(
                                sda_t[:, r * 128 * lv["nb"]:
                                      r * 128 * lv["nb"] + lv["s"]], sla)
                        else:
                            nc.scalar.copy(nxt_rows[:, r, 1:1 + lv["s"]], sla)
                    r0 += nr
                out_t = packed.tile([128, rows * 128 * lv["nb"]], F32,
                                    tag=f"outp{li}")
                outa_t = None
                if last:
                    outa_t = packed.tile([128, rows * 128 * lv["nb"]], F32,
                                         tag="outpa")
                for r in range(rows):
                    base = r * 128 * lv["nb"]
                    emit_out(sd_t, out_t, lv, base=base)
                    if last:
                        emit_out(sda_t, outa_t, lv, base=base)
                    dma_out(out_t, d_out[li], r, lv, base_col=base)
                    if last:
                        dma_out(outa_t, a6_out, r, lv, base_col=base)
                if not last:
                    cur = nxt
    nc.compile()
    return nc


_CACHE = {}


def _get_nc():
    if "nc" not in _CACHE:
        _CACHE["nc"] = build_nc()
        _CACHE["w"] = make_band_arrays()
    return _CACHE["nc"], _CACHE["w"]


LAST_RESULT = None


def kernel(x):
    global LAST_RESULT
    x = np.ascontiguousarray(np.asarray(x), dtype=np.float32)
    assert x.shape == (B_FULL, N0)
    from concourse.bass_utils import run_bass_kernel_spmd

    nc, (wmain, whalo) = _get_nc()
    in_maps = [
        {"x": x[c * ROWS:(c + 1) * ROWS], "wmain": wmain, "whalo": whalo}
        for c in range(N_CORES)
    ]
    res = run_bass_kernel_spmd(nc, in_maps, core_ids=list(range(N_CORES)))
    LAST_RESULT = res
    outs = []
    for name in OUT_NAMES:
        outs.append(np.concatenate([res.results[c][name]
                                    for c in range(N_CORES)], axis=0))
    return tuple(outs)
